# revision 7
# baseline (speedup 1.0000x reference)
# GNN mean-aggregation kernel for Trainium2 (8 NeuronCores, SPMD). v9
#
# Computes: out[i] = (1/deg_i) * sum_{(i,j) in E} (x[j] @ W + b)
# using the identity  out = inv_deg * (A @ x) @ W + b*mask, with the
# inv_deg scale and bias applied on the host (elementwise epilogue).
#
# Sharding: destination nodes (and their incoming edge rows -- `row` is
# sorted) are split contiguously across 8 cores; x and W are replicated,
# so no collectives are needed.
#
# Key structure (v9, ~1.09ms vs 1.23ms v6 baseline):
#   - x gathered in bf16 via SWDGE dma_gather (1024-idx calls: ucode cap;
#     64KB descriptor scratch so desc-gen streams ahead of the drain).
#   - one-hot segment matrices built on DVE in 2x_1p mode (packed bf16,
#     layout st[edge, dest, col], broadcast on the middle dim).
#   - PSUM accumulation is per (tile, chunk) with start/stop, folded into
#     a per-group SBUF aggregate (Act copy on chunk 0, DVE add after), so
#     PSUM banks don't cap the group size: G=10 tiles/group -> 40
#     (group, chunk) iterations instead of 68, amortizing per-iteration
#     serialization that dominates on HW.
#   - epilogue: 512-wide W matmuls straight off the SBUF aggregate.

import math
import os

# harness may call kernel() directly in a fresh process; the bass->PJRT
# execution path needs the axon jax platform registered before jax import
os.environ.setdefault("JAX_PLATFORMS", "axon,cpu")

import numpy as np

P = 128
F = 128


class _Cfg:
    def __init__(self, n_nodes, n_cores, n_chunks, group_tiles=8):
        self.NN = n_nodes
        self.NCORES = n_cores
        self.NDEST = n_nodes // n_cores
        self.NT = math.ceil(self.NDEST / P)
        self.NCH = n_chunks
        self.CH = math.ceil(n_nodes / n_chunks)
        assert self.CH <= 32768
        self.G = group_tiles


# v8: G=8 tiles (1024 psum cols = 2 banks) accumulate across all 4
# chunks directly in PSUM (per-bank start/stop); the per-(tile,chunk)
# fold chain is gone.  acc 2 banks x2 bufs + wb 1 bank x2 = 6 of 8.
CFG = _Cfg(100000, 8, 4, group_tiles=8)

_BUILD_CACHE = {}

# max indices per dma_gather call: the SWDGE ucode caps num_idxs at 1024
# per call (2048+ crashes).  dynamic_dma_scratch_size=65536 gives each of
# the 4 SWDGE queues a 4096-descriptor ring so descriptor generation can
# run ahead of the DMA drain (measured ~14% faster than the default 16KB
# ring, which fits exactly one call and stalls desc-gen on reclaim).
GCALL = 1024
DMA_SCRATCH = 65536
IOTA_COLS = 48  # max cols per one-hot DVE call


def _groups(cfg):
    return [(t0, min(t0 + cfg.G, cfg.NT)) for t0 in range(0, cfg.NT, cfg.G)]


def _host_prep(cfg, x, row, col, W, b):
    NN, NCORES, NDEST, NT, NCH, CH = (
        cfg.NN, cfg.NCORES, cfg.NDEST, cfg.NT, cfg.NCH, cfg.CH)
    NE = row.shape[0]
    row = np.asarray(row).astype(np.int64)
    col = np.asarray(col).astype(np.int64)
    x = np.ascontiguousarray(np.asarray(x, dtype=np.float32))
    W = np.ascontiguousarray(np.asarray(W, dtype=np.float32))
    b = np.asarray(b, dtype=np.float32)

    deg = np.bincount(row, minlength=NN).astype(np.float32)
    invdeg = np.where(deg > 0, 1.0 / np.maximum(deg, 1.0), 0.0).astype(np.float32)

    core = row // NDEST
    r_in_core = row % NDEST
    chunk = col // CH
    idx16 = (col % CH).astype(np.int16)

    # Natural (contiguous) dest->tile assignment unless some (tile, chunk)
    # bin would push C_sub above 9 columns; then greedily rebalance.
    nat_tile = r_in_core // P
    nat_key = (core * NT + nat_tile) * NCH + chunk
    nat_max = np.bincount(nat_key, minlength=NCORES * NT * NCH).max()
    if nat_max <= 9 * P:
        perm = np.tile(np.arange(NDEST, dtype=np.int64)[None, :], (NCORES, 1))
        tilei = nat_tile
        rel = (r_in_core % P).astype(np.float32)
    else:
        # perm[core, d_local] = permuted position (tile*128 + slot).
        perm = np.zeros((NCORES, NDEST), np.int64)
        for c in range(NCORES):
            cnt = np.zeros((NDEST, NCH), np.int32)
            np.add.at(cnt, (r_in_core[core == c], chunk[core == c]), 1)
            order_d = np.argsort(-cnt.max(axis=1), kind="stable")
            sums = np.zeros((NT, NCH), np.int32)
            counts = np.zeros(NT, np.int32)
            pos = np.empty(NDEST, np.int64)
            big = np.int32(1 << 30)
            for d in order_d:
                newmax = np.maximum(sums, cnt[d]).max(axis=1)
                t = int(np.argmin(np.where(counts < P, newmax, big)))
                pos[d] = t * P + counts[t]
                counts[t] += 1
                sums[t] += cnt[d]
            perm[c] = pos
        tilei = perm[core, r_in_core] // P
        rel = (perm[core, r_in_core] % P).astype(np.float32)

    bin_key = (core * NT + tilei) * NCH + chunk
    nbins = NCORES * NT * NCH
    counts = np.bincount(bin_key, minlength=nbins)
    C_sub = max(1, int(math.ceil(counts.max() / P)))
    SLOT = C_sub * P

    order = np.argsort(bin_key, kind="stable")
    sk = bin_key[order]
    starts = np.concatenate([[0], np.cumsum(counts)[:-1]])
    rank = np.arange(NE, dtype=np.int64) - starts[sk]
    pos = sk * SLOT + rank

    TOT = nbins * SLOT
    idx_pad = np.zeros(TOT, np.int16)
    rel_pad = np.full(TOT, -1.0, np.float32)
    idx_pad[pos] = idx16[order]
    rel_pad[pos] = rel[order]
    idx_pad = idx_pad.reshape(NCORES, NT, NCH, SLOT)
    rel_pad = rel_pad.reshape(NCORES, NT, NCH, C_sub, P)

    groups = _groups(cfg)

    # iota_full[p, q, k] = q, bf16, for the packed one-hot compare
    # (one-hot is built in <=IOTA_COLS-wide calls so the const stays small)
    GC = IOTA_COLS
    iota2 = np.tile(
        np.arange(P, dtype=np.float32)[None, :, None], (P, 1, GC)
    ).reshape(P, P * GC)
    iota2 = _to_bf16(iota2)

    x_bf = _to_bf16(x)

    in_maps = []
    for c in range(NCORES):
        # gather-call index stream + rel columns, in device consumption
        # order: per (group, chunk), wrapped per <=GCALL-idx call
        wrapped_parts = []
        rel_parts = []
        for (t0, t1) in groups:
            for ch in range(NCH):
                seq = idx_pad[c, t0:t1, ch].reshape(-1)
                for k0 in range(0, len(seq), GCALL):
                    seg = seq[k0:k0 + GCALL]
                    wrapped_parts.append(
                        np.tile(seg.reshape(-1, 16).T, (8, 1)))
                # [gt, C_sub, P] -> [gt*C_sub, P]
                rel_parts.append(rel_pad[c, t0:t1, ch].reshape(-1, P))
        idx_t = np.concatenate(wrapped_parts, axis=1)
        rel_t = _to_bf16(np.ascontiguousarray(np.concatenate(rel_parts, axis=0).T))

        in_maps.append({
            "x": x_bf,
            "idxs": np.ascontiguousarray(idx_t),
            "rel": rel_t,
            "w": W,
            "iota2": iota2,
        })

    host_post = {
        "perm": perm,
        "invdeg": invdeg,
        "deg": deg,
        "b": b,
    }
    return C_sub, in_maps, host_post


def _to_bf16(x32):
    """float32 -> bfloat16 (round-to-nearest-even) as uint16 view with
    ml_dtypes if available, else manual rounding."""
    try:
        import ml_dtypes
        return x32.astype(ml_dtypes.bfloat16)
    except ImportError:
        u = x32.view(np.uint32)
        rounded = (u + 0x7FFF + ((u >> 16) & 1)) >> 16
        return rounded.astype(np.uint16)


def _build(cfg, C_sub, repeat, parts=("gather", "onehot", "mm"), x_f32=False):
    import concourse.mybir as mybir
    import concourse.tile as tile
    from concourse import bacc

    f32 = mybir.dt.float32
    bf16 = mybir.dt.bfloat16
    i16 = mybir.dt.int16
    eq = mybir.AluOpType.is_equal

    NT, NCH, CH, G = cfg.NT, cfg.NCH, cfg.CH, cfg.G
    C_tot = NCH * C_sub
    IDXW = NT * C_tot * P // 16
    GC = IOTA_COLS

    nc = bacc.Bacc("TRN2", debug=False, num_swdge_queues=4,
                   dynamic_dma_scratch_size=DMA_SCRATCH)
    x_dt = f32 if x_f32 else bf16
    x_d = nc.dram_tensor("x", [cfg.NN, F], x_dt, kind="ExternalInput")
    idx_d = nc.dram_tensor("idxs", [P, IDXW], i16, kind="ExternalInput")
    rel_d = nc.dram_tensor("rel", [P, NT * C_tot], bf16, kind="ExternalInput")
    w_d = nc.dram_tensor("w", [F, F], f32, kind="ExternalInput")
    iota_d = nc.dram_tensor("iota2", [P, P * GC], bf16, kind="ExternalInput")
    out_d = nc.dram_tensor("outT", [P, NT * P], f32, kind="ExternalOutput")

    groups = _groups(cfg)
    x_ap = x_d.ap()

    with tile.TileContext(nc) as tc:
        with (
            tc.tile_pool(name="const", bufs=1) as constp,
            tc.tile_pool(name="reg", bufs=3) as regionp,
            tc.tile_pool(name="st", bufs=3) as stp,
            tc.tile_pool(name="gidx", bufs=2) as gidxp,
            tc.tile_pool(name="agg", bufs=2) as aggp,
            tc.tile_pool(name="small", bufs=2) as smallp,
            tc.tile_pool(name="acc", bufs=2, space="PSUM") as accp,
            tc.tile_pool(name="wb", bufs=2, space="PSUM") as wbp,
        ):
            w_sb = constp.tile([F, F], f32)
            nc.sync.dma_start(w_sb[:], w_d.ap())
            iota_sb = constp.tile([P, P, GC], bf16)
            nc.sync.dma_start(iota_sb[:], iota_d.ap())
            rel_sb = constp.tile([P, NT * C_tot], bf16)
            nc.sync.dma_start(rel_sb[:], rel_d.ap())

            def body(_iv=None):
                idx_off = 0
                rel_off = 0
                qn = 0
                for (t0, t1) in groups:
                    gt = t1 - t0
                    # v9: stage the whole group's idx stream with ONE DMA
                    # (covers 4 chunks = 36 gather calls), bufs=2 so the
                    # next group's idxs load during this group's gathers
                    gidx_w = NCH * gt * C_sub * 8
                    idxt = gidxp.tile([P, gidx_w], i16, tag="gidx")
                    nc.sync.dma_start(
                        idxt[:], idx_d.ap()[:, idx_off:idx_off + gidx_w])
                    gidx_off = 0
                    # psum accumulation group spans all 4 chunks; groups
                    # are per 2KB bank (512 f32 cols = 4 tiles of 128)
                    TPB = 4
                    accap = None
                    if "mm" in parts:
                        acc = accp.tile([P, gt * P], f32, tag="acc")
                        accap = acc[:]
                    nbank = (gt + TPB - 1) // TPB
                    bank_first = {}
                    bank_last = {}
                    for c in range(NCH):
                        for ti in range(gt):
                            bk = ti // TPB
                            for j in range(C_sub):
                                if bk not in bank_first:
                                    bank_first[bk] = (c, ti, j)
                                bank_last[bk] = (c, ti, j)
                    for c in range(NCH):
                        ncols = gt * C_sub
                        reg = regionp.tile([P, ncols, P], x_dt, tag="reg")
                        if "gather" not in parts:
                            nc.vector.memset(reg[:, 0:1, :], 0)
                        gc = GCALL // P  # cols per gather call
                        for k0 in range(0, ncols, gc) if "gather" in parts else []:
                            kc = min(gc, ncols - k0)
                            L = kc * P
                            nc.gpsimd.dma_gather(
                                out_ap=reg[:, k0:k0 + kc, :],
                                in_ap=x_ap[c * CH:min((c + 1) * CH, cfg.NN), :],
                                idxs_ap=idxt[:, gidx_off + k0 * 8:
                                             gidx_off + (k0 + kc) * 8],
                                num_idxs=L,
                                num_idxs_reg=L,
                                elem_size=F,
                                queue_num=qn % 4,
                            )
                            qn += 1
                        gidx_off += ncols * 8
                        idx_off += ncols * 8
                        # st[p=edge, q=dest, k=col] -- last dim packed so the
                        # DVE is_equal runs in 2x_1p mode (all operands 2-byte
                        # with unit-stride last dim); built in <=IOTA_COLS
                        # slices to keep the iota constant small
                        st = stp.tile([P, P, ncols], bf16, tag="st")
                        if "onehot" in parts:
                            for s0 in range(0, ncols, IOTA_COLS):
                                s1 = min(s0 + IOTA_COLS, ncols)
                                nc.vector.tensor_tensor(
                                    out=st[:, :, s0:s1],
                                    in0=iota_sb[:, :, :s1 - s0],
                                    in1=rel_sb[:, rel_off + s0:rel_off + s1]
                                    .unsqueeze(1).to_broadcast(
                                        [P, P, s1 - s0]),
                                    op=eq,
                                )
                        rel_off += ncols
                        for ti in range(gt) if "mm" in parts else []:
                            bk = ti // TPB
                            w0 = ti * P
                            for j in range(C_sub):
                                k = ti * C_sub + j
                                nc.tensor.matmul(
                                    out=accap[:, w0:w0 + P],
                                    lhsT=reg[:, k, :],
                                    rhs=st[:, :, k],
                                    start=(bank_first[bk] == (c, ti, j)),
                                    stop=(bank_last[bk] == (c, ti, j)),
                                )
                    if "mm" not in parts:
                        # ablation: consume a sliver of each reg/st so the
                        # gathers/onehots are kept live and flow to the output
                        osb = smallp.tile([P, P], f32, tag="osb")
                        if "gather" in parts:
                            nc.scalar.copy(osb[:, 0:P], reg[:, 0, :])
                        elif "onehot" in parts:
                            nc.scalar.copy(osb[:, 0:P], st[:, :, 0])
                        else:
                            nc.vector.memset(osb[:], 0)
                        nc.sync.dma_start(
                            out_d.ap()[:, t0 * P:(t0 + 1) * P], osb[:])
                        continue
                    # epilogue: psum -> sbuf agg, then wide W matmuls
                    agg = aggp.tile([P, gt * P], f32, tag="agg")
                    nc.scalar.copy(agg[:], accap)
                    for w0 in range(0, gt * P, 512):
                        w1 = min(w0 + 512, gt * P)
                        wb = wbp.tile([P, w1 - w0], f32, tag="wb")
                        nc.tensor.matmul(out=wb[:], lhsT=w_sb[:],
                                         rhs=agg[:, w0:w1],
                                         start=True, stop=True)
                        osb = smallp.tile([P, w1 - w0], f32, tag="osb")
                        nc.scalar.copy(osb[:], wb[:])
                        nc.sync.dma_start(
                            out_d.ap()[:, t0 * P + w0:t0 * P + w1], osb[:])

            if repeat == 1:
                body()
            else:
                with tc.For_i(0, repeat, 1) as iv:
                    body(iv)

    nc.compile()
    return nc


def _run(cfg, x, row, col, W, b, repeat=1, core_ids=None):
    from concourse import bass_utils

    C_sub, in_maps, post = _host_prep(cfg, x, row, col, W, b)
    key = (cfg.NN, cfg.NCORES, C_sub, repeat)
    if key not in _BUILD_CACHE:
        _BUILD_CACHE[key] = _build(cfg, C_sub, repeat)
    nc = _BUILD_CACHE[key]
    if core_ids is None:
        core_ids = list(range(cfg.NCORES))
    res = bass_utils.run_bass_kernel_spmd(nc, in_maps, core_ids=core_ids)
    outs = []
    for c in range(len(core_ids)):
        outT = res.results[c]["outT"]
        outs.append(outT.T[post["perm"][c]])
    out = np.concatenate(outs, axis=0)
    out = out * post["invdeg"][:, None] \
        + post["b"][None, :] * (post["deg"] > 0)[:, None]
    return np.ascontiguousarray(out, dtype=np.float32)


def kernel(x, row, col, W, b):
    return _run(CFG, x, row, col, W, b, repeat=1)

